# revision 1
# baseline (speedup 1.0000x reference)
"""Bilateral filter (d=9, sigmaColor=sigmaSpace=1.5) on 8 Trainium2 NeuronCores.

Contract: kernel(img: np.ndarray[3,1024,1024] f32) -> np.ndarray[3,1024,1024] f32.

Strategy (measured ~224 us/core steady-state, rel err ~2.6e-4):
  - Host reflect-pads the full image by r=4 on H and W, then shards H across
    8 cores with a 4-row halo on each side (rows i*128 .. i*128+136 of the
    padded image).  No device-to-device communication is needed.
  - Each core holds a fp16 "strip" per channel in SBUF: partition p (= output
    row p) stores padded rows p..p+8 in the free dimension, so every tap
    offset (dy, dx) is a pure free-dim AP offset — zero on-chip data
    movement.  A one-element-shifted copy (B) serves odd-dx taps (4-byte
    alignment keeps the DVE tensor_tensor ops in 2x fp16 mode).
  - Per tap the vector engine does sub / L1 channel-sum / product only.  The
    scalar engine evaluates abs, Square and Exp batched over symmetric tap
    pairs (same r^2 = dy^2+dx^2 -> shared exp bias; the spatial weight is
    folded in: w = exp(-(d1^2 + r^2) / (2*sigma^2))).
  - All accumulation runs on the otherwise-idle TensorEngine: identity-matmuls
    accumulate each tap's w*S (and w for the denominator) into fp32 PSUM, so
    accumulation is both free and carries no fp16 rounding.  The centre tap
    (w == 1 exactly) opens the PSUM accumulation groups.
  - Output: num * reciprocal_approx(den) in fp32, DMA'd back permuted.
"""

import sys

for _p in ("/opt/trn_rl_repo",):
    if _p not in sys.path:
        sys.path.insert(0, _p)

import numpy as np

import concourse.bass as bass  # noqa: F401  (registers engine classes)
import concourse.tile as tile
from concourse import bacc, mybir
from concourse.bass_utils import run_bass_kernel_spmd

C, H, W = 3, 1024, 1024
R = 4  # kernel radius (d=9)
STD = 1.5
INV2S2 = 0.5 / (STD * STD)
N_CORES = 8
HSH = H // N_CORES  # 128 output rows per core
PW = W + 2 * R  # padded width 1032
ROWS = 2 * R + 1  # 9 rows per strip
RSTRIDE = PW + 4  # strip row stride (1036, 4B*4-aligned padding)

OFFSETS = [
    (dy, dx)
    for dy in range(-R, R + 1)
    for dx in range(-R, R + 1)
    if dy * dy + dx * dx <= R * R
]  # 49 taps

F32 = mybir.dt.float32
F16 = mybir.dt.float16
ALU = mybir.AluOpType
ACTF = mybir.ActivationFunctionType

# All 48 non-centre taps form 24 symmetric pairs sharing r^2 = dy^2+dx^2
# (the centre tap has w == 1 exactly and is folded into the PSUM group init).
# The ACT unary ops (abs/square/exp) of a pair are batched into single
# instructions — same r^2 means the exp bias is shared.  DVE ops keep their
# per-tap AP shapes: batching taps into DVE ops via a stride-2 AP dim was
# measured slower on HW (suspected 2x fp16 mode fallback; the cost model
# cannot see it).
# ordered so pairs touching rows nearest the centre come first, matching the
# DMA load order (shortens the startup critical path)
PAIRS = []
for _ady in range(0, R + 1):
    for _dy in ((0,) if _ady == 0 else (-_ady, _ady)):
        for _dx in range(1, R + 1):
            if _dy * _dy + _dx * _dx <= R * R:
                PAIRS.append(("dx", _dy, _dx))
    if _ady >= 1:
        PAIRS.append(("dy", _ady, 0))


def _build_program_fp16(reps: int = 1):
    """fp16 compute pipeline: DVE tensor_tensor ops run in 2x mode (16-bit,
    unit-stride, 4B-aligned).  Odd dx offsets are 2-byte aligned, so a
    one-element-shifted copy (B) of the strip serves those taps.

    Accumulation runs on the TensorEngine: identity-matmuls accumulate each
    tap's w*S products (and w itself for the denominator) into PSUM in fp32,
    so the vector engine only does sub / L1-sum / product per tap, and the
    accumulation carries no fp16 rounding."""
    nc = bacc.Bacc(
        "TRN2", target_bir_lowering=False, debug=False, num_devices=N_CORES
    )
    x = nc.dram_tensor("x", [C, HSH + 2 * R, PW], F16, kind="ExternalInput").ap()
    ident = nc.dram_tensor("ident", [128, 128], F16, kind="ExternalInput").ap()
    y = nc.dram_tensor("y", [C, HSH, W], F32, kind="ExternalOutput").ap()

    with tile.TileContext(nc) as tc:
        with (
            tc.tile_pool(name="strips", bufs=1) as strip_pool,
            tc.tile_pool(name="accs", bufs=1) as acc_pool,
            tc.tile_pool(name="tmps", bufs=2) as tmp_pool,
            tc.tile_pool(name="psum", bufs=1, space="PSUM") as psum_pool,
        ):
            # Load order matters for the startup critical path: the centre row
            # (4) feeds every tap's subtract, and the first pairs consume rows
            # closest to the centre — load row-major in consumption order,
            # interleaved across channels, with the shifted B rows in between.
            A = strip_pool.tile([128, C, ROWS, RSTRIDE], F16, name="stripA")
            # B = strip shifted one column left (for odd-dx taps, 4B
            # alignment); only dy in [-3, 3] has odd-dx taps -> 7 rows.
            # Loaded straight from DRAM, not copied from A.
            B = strip_pool.tile([128, C, 7, RSTRIDE], F16, name="stripB")
            for j in (4, 3, 5, 2, 6, 1, 7, 0, 8):
                for c in range(C):
                    nc.sync.dma_start(A[:, c, j, 0:PW], x[c, j : j + HSH, :])
                if 1 <= j <= 7:  # B row (j-1) mirrors A row j
                    for c in range(C):
                        nc.sync.dma_start(
                            B[:, c, j - 1, 0 : PW - 1], x[c, j : j + HSH, 1:PW]
                        )

            idt = acc_pool.tile([128, 128], F16, name="idt")
            nc.sync.dma_start(idt[:], ident[:])
            ones = acc_pool.tile([128, W], F16, name="ones")
            nc.vector.memset(ones[:], 1.0)

            # fp32 PSUM accumulators: 6 banks for num, 2 for den (16 KiB exactly)
            num_ps = psum_pool.tile([128, C, W], F32, name="num_ps")
            den_ps = psum_pool.tile([128, W], F32, name="den_ps")

            r2s = sorted({dy * dy + dx * dx for dy, dx in OFFSETS})
            bias_tiles = {}
            for r2 in r2s:
                b = acc_pool.tile([128, 1], F32, tag=f"bias{r2}", name=f"bias{r2}")
                nc.gpsimd.memset(b[:], -float(r2) * INV2S2)
                bias_tiles[r2] = b

            def S(dy, dx):
                if dx % 2 == 0:
                    return A[:, :, R + dy, R + dx : R + dx + W]
                return B[:, :, dy + 3, R - 1 + dx : R - 1 + dx + W]

            C0 = A[:, :, R, R : R + W]

            HB = W // 512  # 512-wide bank slices per channel row

            def acc_num(rhs3, start, stop):
                # num_ps[:, c, h*512:+512] (+)= I.T @ rhs3[:, c, h*512:+512]
                for c in range(C):
                    for h in range(HB):
                        nc.tensor.matmul(
                            num_ps[:, c, h * 512 : (h + 1) * 512],
                            idt[:],
                            rhs3[:, c, h * 512 : (h + 1) * 512],
                            start=start, stop=stop,
                        )

            def acc_den(rhs1, start, stop):
                for h in range(HB):
                    nc.tensor.matmul(
                        den_ps[:, h * 512 : (h + 1) * 512],
                        idt[:],
                        rhs1[:, h * 512 : (h + 1) * 512],
                        start=start, stop=stop,
                    )

            for _rep in range(reps):
                # The centre tap (0,0) has w == 1 exactly (d1 = 0, r^2 = 0):
                # it opens the PSUM accumulation groups (start=True resets).
                acc_num(C0, start=True, stop=False)
                acc_den(ones[:], start=True, stop=False)

                for pi, (kind, dy, dx) in enumerate(PAIRS):
                    last_pair = pi == len(PAIRS) - 1
                    r2 = dy * dy + dx * dx
                    taps = (
                        [(dy, -dx), (dy, dx)] if kind == "dx"
                        else [(-dy, 0), (dy, 0)]
                    )
                    dd2 = tmp_pool.tile(
                        [128, 2, C, W], F16, tag="dd2", name="dd2", bufs=2
                    )
                    for t, (tdy, tdx) in enumerate(taps):
                        nc.vector.tensor_sub(dd2[:, t], S(tdy, tdx), C0)
                    nc.scalar.activation(dd2[:], dd2[:], ACTF.Abs)
                    d1p = tmp_pool.tile(
                        [128, 2, W], F16, tag="d1p", name="d1p", bufs=2
                    )
                    # per-tap csum: batching both taps into [128,2,W] DVE ops
                    # measured slower on HW (240us vs 222us) — DVE tensor ops
                    # stay per-tap, only ACT unaries are pair-batched.
                    for t in range(2):
                        nc.vector.tensor_add(
                            d1p[:, t], dd2[:, t, 0, :], dd2[:, t, 1, :]
                        )
                        nc.vector.tensor_add(
                            d1p[:, t], d1p[:, t], dd2[:, t, 2, :]
                        )
                    q2 = tmp_pool.tile([128, 2, W], F32, tag="q2", name="q2", bufs=1)
                    nc.scalar.activation(q2[:], d1p[:], ACTF.Square)
                    w2 = tmp_pool.tile([128, 2, W], F16, tag="w2", name="w2", bufs=2)
                    nc.scalar.activation(
                        w2[:], q2[:], ACTF.Exp,
                        bias=bias_tiles[r2][:], scale=-INV2S2,
                    )
                    for t, (tdy, tdx) in enumerate(taps):
                        wb = w2[:, t].unsqueeze(1).broadcast_to([128, C, W])
                        p3 = tmp_pool.tile(
                            [128, C, W], F16, tag="p3", name="p3", bufs=3
                        )
                        nc.vector.tensor_mul(p3[:], wb, S(tdy, tdx))
                        stop = last_pair and t == 1
                        acc_num(p3, start=False, stop=stop)
                        acc_den(w2[:, t], start=False, stop=stop)

            recip = tmp_pool.tile([128, W], F32, tag="recip", name="recip", bufs=1)
            scratch = tmp_pool.tile([128, W], F32, tag="q2", name="rscratch", bufs=1)
            # den in [1, 49]: no reciprocal edge cases; ~2 ULP is plenty here
            nc.vector.reciprocal_approx_accurate(recip[:], den_ps[:], scratch[:])
            rb = recip[:].unsqueeze(1).broadcast_to([128, C, W])
            o32 = tmp_pool.tile([128, C, W], F32, name="o32", bufs=1)
            nc.vector.tensor_mul(o32[:], num_ps[:], rb)
            nc.sync.dma_start(y.rearrange("c p x -> p c x"), o32[:])

    nc.compile()
    return nc


def _build_program(reps: int = 1):
    """Build + compile the single-core SPMD Bass program."""
    nc = bacc.Bacc(
        "TRN2", target_bir_lowering=False, debug=False, num_devices=N_CORES
    )
    x = nc.dram_tensor("x", [C, HSH + 2 * R, PW], F32, kind="ExternalInput").ap()
    y = nc.dram_tensor("y", [C, HSH, W], F32, kind="ExternalOutput").ap()

    with tile.TileContext(nc) as tc:
        with (
            tc.tile_pool(name="strips", bufs=1) as strip_pool,
            tc.tile_pool(name="accs", bufs=1) as acc_pool,
            tc.tile_pool(name="tmps", bufs=2) as tmp_pool,
        ):
            strips = []
            for c in range(C):
                s = strip_pool.tile([128, ROWS, RSTRIDE], F32, tag=f"strip{c}", name=f"strip{c}")
                for j in range(ROWS):
                    nc.sync.dma_start(s[:, j, 0:PW], x[c, j : j + HSH, :])
                strips.append(s)

            num = [acc_pool.tile([128, W], F32, tag=f"num{c}", name=f"num{c}") for c in range(C)]
            den = acc_pool.tile([128, W], F32, tag="den")

            # Per-partition bias constants for the fused exp:
            # w = exp(-(d1^2 + r^2) * INV2S2)  -> bias = -r^2 * INV2S2
            r2s = sorted({dy * dy + dx * dx for dy, dx in OFFSETS})
            bias_tiles = {}
            for r2 in r2s:
                b = acc_pool.tile([128, 1], F32, tag=f"bias{r2}", name=f"bias{r2}")
                nc.gpsimd.memset(b[:], -float(r2) * INV2S2)
                bias_tiles[r2] = b

            def S(c, dy, dx):
                return strips[c][:, R + dy, R + dx : R + dx + W]

            for _rep in range(reps):
                for t in num:
                    nc.vector.memset(t[:], 0.0)
                nc.vector.memset(den[:], 0.0)

                for dy, dx in OFFSETS:
                    # diffs, channel-interleaved: dd[p, x, c] = S_c(dy,dx) - C_c
                    dd = tmp_pool.tile([128, W, C], F32, tag="dd", name="dd")
                    for c in range(C):
                        nc.vector.tensor_sub(dd[:, :, c], S(c, dy, dx), S(c, 0, 0))
                    # d1 = sum_c |dd|  (L1 color distance) in one reduce
                    d1 = tmp_pool.tile([128, W], F32, tag="d1", name="d1")
                    nc.vector.tensor_reduce(
                        d1[:], dd[:], mybir.AxisListType.X, ALU.add,
                        apply_absolute_value=True,
                    )
                    # w = exp(-(d1^2 + r^2) * INV2S2), spatial weight folded in
                    w = tmp_pool.tile([128, W], F32, tag="w", name="w")
                    nc.scalar.activation(d1[:], d1[:], ACTF.Square)
                    nc.scalar.activation(
                        w[:], d1[:], ACTF.Exp,
                        bias=bias_tiles[dy * dy + dx * dx][:], scale=-INV2S2,
                    )
                    for c in range(C):
                        p = tmp_pool.tile([128, W], F32, tag="p", name="p", bufs=3)
                        nc.vector.tensor_mul(p[:], w[:], S(c, dy, dx))
                        nc.vector.tensor_add(num[c][:], num[c][:], p[:])
                    nc.vector.tensor_add(den[:], den[:], w[:])

            recip = tmp_pool.tile([128, W], F32, tag="recip", name="recip", bufs=1)
            nc.vector.reciprocal(recip[:], den[:])
            for c in range(C):
                o = tmp_pool.tile([128, W], F32, tag="p", name="o", bufs=3)
                nc.vector.tensor_mul(o[:], num[c][:], recip[:])
                nc.sync.dma_start(y[c], o[:])

    nc.compile()
    return nc


import os

IMPL = os.environ.get("BILATERAL_IMPL", "fp16")

_CACHE: dict = {}


def _get_program(reps: int = 1, impl: str | None = None):
    impl = impl or IMPL
    key = (impl, reps)
    if key not in _CACHE:
        build = _build_program_fp16 if impl == "fp16" else _build_program
        _CACHE[key] = build(reps)
    return _CACHE[key]


def _shards(img: np.ndarray, impl: str | None = None) -> list[dict]:
    impl = impl or IMPL
    padded = np.pad(img, ((0, 0), (R, R), (R, R)), mode="reflect")
    if impl == "fp16":
        padded = padded.astype(np.float16)
        ident = np.eye(128, dtype=np.float16)
        return [
            {
                "x": np.ascontiguousarray(
                    padded[:, i * HSH : i * HSH + HSH + 2 * R, :]
                ),
                "ident": ident,
            }
            for i in range(N_CORES)
        ]
    return [
        {"x": np.ascontiguousarray(padded[:, i * HSH : i * HSH + HSH + 2 * R, :])}
        for i in range(N_CORES)
    ]


def kernel(img: np.ndarray) -> np.ndarray:
    img = np.asarray(img, dtype=np.float32)
    assert img.shape == (C, H, W)
    nc = _get_program()
    res = run_bass_kernel_spmd(nc, _shards(img), list(range(N_CORES))).results
    return np.concatenate([res[i]["y"] for i in range(N_CORES)], axis=1)

